# revision 7
# baseline (speedup 1.0000x reference)
"""Tensor-parallel GQA attention forward for one TRN2 chip (8 NeuronCores).

Strategy (8-way tensor parallel over heads):
  - each core owns 4 q-heads + 1 kv-head (wq/wk/wv column-sharded, host side)
  - x is transposed on-device: each core PE-transposes its 256-row slice of x
    (cast to bf16) and an AllGather assembles the full xT on every core
  - projections produce qT/kT (head_dim on partitions) and v (natural layout)
    directly in the layouts the attention matmuls want; RoPE is applied in a
    de-interleaved head-dim ordering (dot products are permutation invariant)
  - scores are computed transposed (S^T[k, q]) so exp runs straight out of
    PSUM; softmax denominators come for free as a 65th column of ones in the
    PV matmul; causal masking = skipping k-tiles above the diagonal plus a
    0/1 pattern multiply on the 4 diagonal-band tiles per chunk
  - an AllToAll flips head-sharded attnT to sequence-sharded, each core then
    computes its 256-row slice of the output projection against full wo
  - compute dtype bf16 (fp32 PSUM accumulation), output fp32
"""

import numpy as np

NC_CORES = 8
SEQ = 2048
DIM = 2048
HD = 64            # head dim
LHEADS = 4         # q heads per core
SC = SEQ // NC_CORES   # 256: sequence rows per core (transpose shard / output shard)
CH = 512           # q-chunk width for attention
NCH = SEQ // CH    # 4
KT = SEQ // 128    # 16 k-tiles
DT = DIM // 128    # 16 d-tiles

_CACHE = {}


def _build_nc():
    import concourse.bass as bass
    import concourse.mybir as mybir
    import concourse.tile as tile
    from concourse import bacc
    from concourse.masks import make_identity

    BF = mybir.dt.bfloat16
    F32 = mybir.dt.float32
    MUL = mybir.AluOpType.mult
    ADD = mybir.AluOpType.add

    nc = bacc.Bacc("TRN2", target_bir_lowering=False, debug=False,
                   num_devices=NC_CORES)

    # ---- external I/O (per-core shards) ----
    x_sl = nc.dram_tensor("x_sl", [SC, DIM], F32, kind="ExternalInput")
    wqp = nc.dram_tensor("wqp", [DIM, LHEADS * HD], F32, kind="ExternalInput")
    wkv = nc.dram_tensor("wkv", [DIM, 2 * HD], F32, kind="ExternalInput")
    wo = nc.dram_tensor("wo", [DIM, DIM], F32, kind="ExternalInput")
    cosT = nc.dram_tensor("cosT", [32, SEQ], F32, kind="ExternalInput")
    sinT = nc.dram_tensor("sinT", [32, SEQ], F32, kind="ExternalInput")
    out = nc.dram_tensor("out", [SC, DIM], F32, kind="ExternalOutput")

    groups = [list(range(NC_CORES))]

    with tile.TileContext(nc) as tc:
        # DRAM bounce buffers for collectives
        ag_in, _ = tc.tile([DT, 128, SC], BF, space=bass.MemorySpace.DRAM,
                           name="ag_in")
        ag_out, _ = tc.tile([NC_CORES, DT, 128, SC], BF,
                            space=bass.MemorySpace.DRAM,
                            addr_space="Shared", name="ag_out")
        a2a_in, _ = tc.tile([NC_CORES, 2 * 128, SC], BF,
                            space=bass.MemorySpace.DRAM, name="a2a_in")
        a2a_out, _ = tc.tile([NC_CORES, 2 * 128, SC], BF,
                             space=bass.MemorySpace.DRAM,
                             addr_space="Shared", name="a2a_out")

        with tc.tile_pool(name="big", bufs=1) as bigp, \
             tc.tile_pool(name="persist", bufs=1) as pp, \
             tc.tile_pool(name="work", bufs=2) as wp, \
             tc.tile_pool(name="psum", bufs=2, space="PSUM") as psp:

            # ---------------- phase 0: setup ----------------
            # first 8 k-tiles of wo: dedicated slot, no deps -> loads immediately
            woA = pp.tile([128, DT // 2, DIM], BF, name="woA")
            for g in range(2):
                nc.gpsimd.dma_start(
                    woA[:, 4 * g:4 * g + 4, :],
                    wo[512 * g:512 * g + 512, :].rearrange("(t p) n -> p t n",
                                                           p=128))

            ident = pp.tile([128, 128], BF, name="ident")
            make_identity(nc, ident[:])

            # 0/1 causal patterns for diagonal-band tiles, paired for the
            # [128, 1024] two-head exp tiles: [pat_t | pat_t]
            patp = []
            for t in range(4):
                pat = pp.tile([128, 2, CH], BF, name=f"pat{t}")
                nc.gpsimd.memset(pat[:], 1.0)
                for half in range(2):
                    nc.gpsimd.affine_select(
                        out=pat[:, half, :], in_=pat[:, half, :],
                        compare_op=mybir.AluOpType.is_ge, fill=0.0,
                        base=-128 * t, channel_multiplier=-1, pattern=[[1, CH]],
                    )
                patp.append(pat)

            wq_sb = pp.tile([128, DT, LHEADS * HD], BF, name="wq_sb")
            nc.gpsimd.dma_start(wq_sb[:], wqp[:].rearrange("(t p) m -> p t m", p=128))
            wkv_sb = pp.tile([128, DT, 2 * HD], BF, name="wkv_sb")
            nc.gpsimd.dma_start(wkv_sb[:], wkv[:].rearrange("(t p) m -> p t m", p=128))

            # cos4: cosT replicated on every 32-partition block
            # sin4s: signed sin table: rows 32b..32b+31 = (-1)^(b+1) * sinT
            cos4 = pp.tile([128, SEQ], BF, name="cos4")
            sin4s = pp.tile([128, SEQ], BF, name="sin4s")
            for b in range(4):
                nc.gpsimd.dma_start(cos4[32 * b:32 * b + 32, :], cosT[:])
                nc.gpsimd.dma_start(sin4s[32 * b:32 * b + 32, :], sinT[:])
            for b in (0, 2):   # negate blocks 0 and 2 (the "real out" rows)
                nc.vector.tensor_scalar_mul(sin4s[32 * b:32 * b + 32, :],
                                            sin4s[32 * b:32 * b + 32, :], -1.0)

            xsl_bf = pp.tile([128, 2, DIM], BF, name="xsl_bf")
            nc.gpsimd.dma_start(xsl_bf[:], x_sl[:].rearrange("(t p) d -> p t d", p=128))

            # ---------------- phase 1: transpose own slice + AllGather ----------------
            xTc = pp.tile([128, DT, SC], BF, name="xTc")
            for pt in range(2):
                for j in range(DT):
                    trp = psp.tile([128, 128], BF, tag="ps", bufs=4, name="trp")
                    nc.tensor.transpose(trp[:], xsl_bf[:, pt, 128 * j:128 * j + 128],
                                        ident[:])
                    nc.vector.tensor_copy(xTc[:, j, 128 * pt:128 * pt + 128], trp[:])
            nc.sync.dma_start(ag_in[:].rearrange("t p s -> p t s"), xTc[:])
            nc.gpsimd.collective_compute(
                "AllGather", mybir.AluOpType.bypass,
                replica_groups=groups, ins=[ag_in.opt()], outs=[ag_out.opt()],
            )
            xt = bigp.tile([128, DT, SEQ], BF, tag="bigslot", name="xt")
            for j in range(NCH):   # per-chunk loads so projections start early
                for i in range(2):
                    o = CH * j + SC * i
                    nc.sync.dma_start(
                        xt[:, :, o:o + SC],
                        ag_out[2 * j + i, :, :, :].rearrange("t p s -> p t s"))

            # ---------------- phase 2: projections + RoPE ----------------
            qT = pp.tile([128, 2, SEQ], BF, name="qT")   # [pair]; even head 0-63, odd 64-127
            kT = pp.tile([128, SEQ], BF, name="kT")      # rows 0-63 kT; 64-127 duplicate
            vT = pp.tile([64, SEQ], BF, name="vT")
            v_sb = pp.tile([128, KT, HD + 1], BF, name="v_sb")
            nc.gpsimd.memset(v_sb[:, :, HD:HD + 1], 1.0)

            def rope(ps, nrows, j, dst):
                # dst rows [32b, 32b+32) for even b are "real" outputs, odd b "imag";
                # ps rows: same blocks hold q_r (even) / q_i (odd) pre-rotation.
                # out = ps*cos4 + cross, cross[b] = ps[b^1 block] * sin4s[b]
                sl = slice(CH * j, CH * j + CH)
                ta = wp.tile([128, CH], F32, tag="ropeA", bufs=2, name="ta")
                cr = wp.tile([128, CH], F32, tag="ropeB", bufs=2, name="cr")
                nc.vector.tensor_tensor(ta[0:nrows, :], ps[0:nrows, :],
                                        cos4[0:nrows, sl], MUL)
                for b in range(0, nrows, 32):
                    o = 32 if b % 64 == 0 else -32   # partner block offset
                    nc.vector.tensor_tensor(cr[b:b + 32, :],
                                            ps[b + o:b + o + 32, :],
                                            sin4s[b:b + 32, sl], MUL)
                nc.vector.tensor_tensor(dst, ta[0:nrows, :], cr[0:nrows, :], ADD)

            def qproj(pair, j):
                psq = psp.tile([128, CH], F32, tag="ps", bufs=4, name="psq")
                for dt in range(DT):
                    nc.tensor.matmul(
                        psq[:], wq_sb[:, dt, 128 * pair:128 * pair + 128],
                        xt[:, dt, CH * j:CH * j + CH],
                        start=(dt == 0), stop=(dt == DT - 1))
                rope(psq, 128, j, qT[0:128, pair, CH * j:CH * j + CH])

            for j in range(NCH):
                pskv = psp.tile([128, CH], F32, tag="ps", bufs=4, name="pskv")
                for dt in range(DT):
                    nc.tensor.matmul(pskv[:], wkv_sb[:, dt, :],
                                     xt[:, dt, CH * j:CH * j + CH],
                                     start=(dt == 0), stop=(dt == DT - 1))
                rope(pskv, 64, j, kT[0:64, CH * j:CH * j + CH])
                nc.vector.tensor_copy(vT[:, CH * j:CH * j + CH], pskv[64:128, :])

            # duplicate kT rows 0-63 -> 64-127 (for row-packed dual matmuls)
            nc.vector.tensor_copy(kT[64:128, :], kT[0:64, :])
            # v natural layout via PE transposes of vT
            for kt in range(KT):
                vtp = psp.tile([128, 64], BF, tag="ps", bufs=4, name="vtp")
                nc.tensor.transpose(vtp[:], vT[:, 128 * kt:128 * kt + 128],
                                    ident[0:64, 0:64])
                nc.vector.tensor_copy(v_sb[:, kt, 0:HD], vtp[:])

            for j in range(NCH):
                qproj(0, j)

            # ---------------- phase 3: attention ----------------
            attnT = pp.tile([128, 2, SEQ], BF, name="attnT")

            def attention(pair, j):
                nkt = 4 * j + 4
                pso0 = psp.tile([HD + 1, CH], F32, tag="ps", bufs=4, name="pso0")
                pso1 = psp.tile([HD + 1, CH], F32, tag="ps", bufs=4, name="pso1")
                qsl = slice(CH * j, CH * j + CH)
                for kt in range(nkt):
                    ks = slice(128 * kt, 128 * kt + 128)
                    sp = psp.tile([128, 2 * CH], F32, tag="spair", bufs=2, name="sp")
                    nc.tensor.matmul(sp[:, 0:CH], kT[0:64, ks], qT[0:64, pair, qsl],
                                     start=True, stop=True)
                    nc.tensor.matmul(sp[:, CH:2 * CH], kT[64:128, ks],
                                     qT[64:128, pair, qsl], start=True, stop=True)
                    ep = wp.tile([128, 2 * CH], BF, tag="exps", bufs=4, name="ep")
                    nc.scalar.activation(ep[:], sp[:],
                                         mybir.ActivationFunctionType.Exp,
                                         scale=0.125)
                    if kt >= 4 * j:
                        pat = patp[kt - 4 * j]
                        nc.vector.tensor_tensor(ep[:], ep[:],
                                                pat[:].rearrange("p a c -> p (a c)"),
                                                MUL)
                    nc.tensor.matmul(pso0[:], v_sb[:, kt, :], ep[:, 0:CH],
                                     start=(kt == 0), stop=(kt == nkt - 1))
                    nc.tensor.matmul(pso1[:], v_sb[:, kt, :], ep[:, CH:2 * CH],
                                     start=(kt == 0), stop=(kt == nkt - 1))
                for h, pso in ((0, pso0), (1, pso1)):
                    rc = wp.tile([1, CH], F32, tag="recip", bufs=2, name="rc")
                    nc.vector.reciprocal(rc[:], pso[HD:HD + 1, :])
                    bc = wp.tile([64, CH], F32, tag="bcast", bufs=2, name="bc")
                    nc.gpsimd.partition_broadcast(bc[:], rc[:])
                    nc.vector.tensor_tensor(
                        attnT[64 * h:64 * h + 64, pair, qsl],
                        pso[0:HD, :], bc[:], MUL)

            # interleave: attention(pair 0) with q projections of pair 1
            for j in range(NCH):
                attention(0, j)
                qproj(1, j)
            # second 8 k-tiles of wo: reuses xt's slot (dead after last qproj)
            woB = bigp.tile([128, DT // 2, DIM], BF, tag="bigslot", name="woB")
            for g in range(2):
                nc.gpsimd.dma_start(
                    woB[:, 4 * g:4 * g + 4, :],
                    wo[1024 + 512 * g:1024 + 512 * g + 512, :]
                    .rearrange("(t p) n -> p t n", p=128))
            for j in range(NCH):
                attention(1, j)

            # ---------------- phase 4: AllToAll + output projection ----------------
            for dst in range(NC_CORES):
                for pair in range(2):
                    nc.sync.dma_start(
                        a2a_in[dst, 128 * pair:128 * pair + 128, :],
                        attnT[:, pair, SC * dst:SC * dst + SC])
            nc.gpsimd.collective_compute(
                "AllToAll", mybir.AluOpType.bypass,
                replica_groups=groups, ins=[a2a_in.opt()], outs=[a2a_out.opt()],
            )
            a2a_sb = pp.tile([128, 2 * NC_CORES, SC], BF, name="a2a_sb")
            for src in range(NC_CORES):
                for dt in range(2):
                    nc.sync.dma_start(
                        a2a_sb[:, 2 * src + dt, :],
                        a2a_out[src, 128 * dt:128 * dt + 128, :])

            for qt in range(2):
                for nch in range(NCH):
                    psf = psp.tile([128, CH], F32, tag="ps", bufs=4, name="psf")
                    nsl = slice(CH * nch, CH * nch + CH)
                    for g in range(2 * NC_CORES):
                        w_ap = (woA[:, g, nsl] if g < DT // 2
                                else woB[:, g - DT // 2, nsl])
                        nc.tensor.matmul(psf[:], a2a_sb[:, g, 128 * qt:128 * qt + 128],
                                         w_ap,
                                         start=(g == 0), stop=(g == 2 * NC_CORES - 1))
                    osb = wp.tile([128, CH], F32, tag="osb", bufs=2, name="osb")
                    nc.vector.tensor_copy(osb[:], psf[:])
                    nc.sync.dma_start(out[128 * qt:128 * qt + 128, nsl], osb[:])

    nc.finalize()
    return nc


def _get_nc():
    if "nc" not in _CACHE:
        _CACHE["nc"] = _build_nc()
    return _CACHE["nc"]


_PERM = np.concatenate([np.arange(0, HD, 2), np.arange(1, HD, 2)])  # de-interleave


def _shard(inputs):
    x = np.ascontiguousarray(inputs["x"][0].astype(np.float32))          # [S, D]
    wq, wk, wv = inputs["wq"], inputs["wk"], inputs["wv"]
    wo = np.ascontiguousarray(inputs["wo"].astype(np.float32))
    cosT = np.ascontiguousarray(inputs["freqs_cos"].T.astype(np.float32))
    sinT = np.ascontiguousarray(inputs["freqs_sin"].T.astype(np.float32))
    in_maps = []
    for c in range(NC_CORES):
        wq_c = wq[:, LHEADS * HD * c:LHEADS * HD * (c + 1)].reshape(DIM, LHEADS, HD)
        wqp = np.ascontiguousarray(wq_c[:, :, _PERM].reshape(DIM, LHEADS * HD)
                                   .astype(np.float32))
        wk_c = wk[:, HD * c:HD * (c + 1)][:, _PERM]
        wv_c = wv[:, HD * c:HD * (c + 1)]
        wkv = np.ascontiguousarray(
            np.concatenate([wk_c, wv_c], axis=1).astype(np.float32))
        in_maps.append({
            "x_sl": np.ascontiguousarray(x[SC * c:SC * (c + 1), :]),
            "wqp": wqp,
            "wkv": wkv,
            "wo": wo,
            "cosT": cosT,
            "sinT": sinT,
        })
    return in_maps


def kernel(**inputs):
    from concourse.bass_utils import run_bass_kernel_spmd

    nc = _get_nc()
    in_maps = _shard(inputs)
    res = run_bass_kernel_spmd(nc, in_maps, core_ids=list(range(NC_CORES)))
    out = np.concatenate([res.results[c]["out"] for c in range(NC_CORES)], axis=0)
    return out[None].astype(np.float32)


# revision 11
# speedup vs baseline: 1.0976x; 1.0976x over previous
"""Tensor-parallel GQA attention forward for one TRN2 chip (8 NeuronCores).

Strategy (8-way tensor parallel over heads):
  - each core owns 4 q-heads + 1 kv-head (wq/wk/wv column-sharded, host side)
  - x is transposed on-device: each core PE-transposes its 256-row slice of x
    (cast to bf16) and an AllGather assembles the full xT on every core
  - projections produce qT/kT (head_dim on partitions) and v (natural layout)
    directly in the layouts the attention matmuls want; RoPE is applied in a
    de-interleaved head-dim ordering (dot products are permutation invariant)
  - scores are computed transposed (S^T[k, q]) so exp runs straight out of
    PSUM; softmax denominators come for free as a 65th column of ones in the
    PV matmul; causal masking = skipping k-tiles above the diagonal plus a
    0/1 pattern multiply on the 4 diagonal-band tiles per chunk
  - an AllToAll flips head-sharded attnT to sequence-sharded, each core then
    computes its 256-row slice of the output projection against full wo
  - compute dtype bf16 (fp32 PSUM accumulation), output fp32
"""

import numpy as np

NC_CORES = 8
SEQ = 2048
DIM = 2048
HD = 64            # head dim
LHEADS = 4         # q heads per core
SC = SEQ // NC_CORES   # 256: sequence rows per core (transpose shard / output shard)
CH = 512           # q-chunk width for attention
NCH = SEQ // CH    # 4
KT = SEQ // 128    # 16 k-tiles
DT = DIM // 128    # 16 d-tiles

_CACHE = {}


def _build_nc():
    import concourse.bass as bass
    import concourse.mybir as mybir
    import concourse.tile as tile
    from concourse import bacc
    from concourse.masks import make_identity

    BF = mybir.dt.bfloat16
    F32 = mybir.dt.float32
    MUL = mybir.AluOpType.mult
    ADD = mybir.AluOpType.add

    nc = bacc.Bacc("TRN2", target_bir_lowering=False, debug=False,
                   num_devices=NC_CORES)

    # ---- external I/O (per-core shards) ----
    x_sl = nc.dram_tensor("x_sl", [SC, DIM], F32, kind="ExternalInput")
    wqp = nc.dram_tensor("wqp", [DIM, LHEADS * HD], F32, kind="ExternalInput")
    wkv = nc.dram_tensor("wkv", [DIM, 2 * HD], F32, kind="ExternalInput")
    wo = nc.dram_tensor("wo", [DIM, DIM], F32, kind="ExternalInput")
    cosT = nc.dram_tensor("cosT", [32, SEQ], F32, kind="ExternalInput")
    sinT = nc.dram_tensor("sinT", [32, SEQ], F32, kind="ExternalInput")
    out = nc.dram_tensor("out", [SC, DIM], F32, kind="ExternalOutput")

    groups = [list(range(NC_CORES))]

    with tile.TileContext(nc) as tc:
        # DRAM bounce buffers for collectives
        ag_in, _ = tc.tile([DT, 128, SC], BF, space=bass.MemorySpace.DRAM,
                           name="ag_in")
        ag_out, _ = tc.tile([NC_CORES, DT, 128, SC], BF,
                            space=bass.MemorySpace.DRAM,
                            addr_space="Shared", name="ag_out")
        a2a_in0, _ = tc.tile([NC_CORES, 128, SC], BF,
                             space=bass.MemorySpace.DRAM, name="a2a_in0")
        a2a_out0, _ = tc.tile([NC_CORES, 128, SC], BF,
                              space=bass.MemorySpace.DRAM,
                              addr_space="Shared", name="a2a_out0")
        a2a_in1, _ = tc.tile([NC_CORES, 128, SC], BF,
                             space=bass.MemorySpace.DRAM, name="a2a_in1")
        a2a_out1, _ = tc.tile([NC_CORES, 128, SC], BF,
                              space=bass.MemorySpace.DRAM,
                              addr_space="Shared", name="a2a_out1")

        with tc.tile_pool(name="big", bufs=1) as bigp, \
             tc.tile_pool(name="persist", bufs=1) as pp, \
             tc.tile_pool(name="work", bufs=2) as wp, \
             tc.tile_pool(name="psum", bufs=2, space="PSUM") as psp:

            # ---------------- phase 0+1: x transpose + AllGather first ----------------
            # (critical path: everything waits on the gathered xT, so the x
            # slice load, PE transposes, bounce store and collective go ahead
            # of every weight load in the DMA queues)
            ident = pp.tile([128, 128], BF, name="ident")
            make_identity(nc, ident[:])

            xsl_bf = pp.tile([128, 2, DIM], BF, name="xsl_bf")
            nc.gpsimd.dma_start(xsl_bf[:], x_sl[:].rearrange("(t p) d -> p t d", p=128))

            xTc = pp.tile([128, DT, SC], BF, name="xTc")
            for pt in range(2):
                for j in range(DT):
                    trp = psp.tile([128, 128], BF, tag="ps", bufs=4, name="trp")
                    nc.tensor.transpose(trp[:], xsl_bf[:, pt, 128 * j:128 * j + 128],
                                        ident[:])
                    nc.vector.tensor_copy(xTc[:, j, 128 * pt:128 * pt + 128], trp[:])
            nc.sync.dma_start(ag_in[:].rearrange("t p s -> p t s"), xTc[:])
            nc.gpsimd.collective_compute(
                "AllGather", mybir.AluOpType.bypass,
                replica_groups=groups, ins=[ag_in.opt()], outs=[ag_out.opt()],
            )
            xt = bigp.tile([128, DT, SEQ], BF, tag="bigslot", name="xt")
            for j in range(NCH):   # per-chunk loads so projections start early
                for i in range(2):
                    o = CH * j + SC * i
                    nc.sync.dma_start(
                        xt[:, :, o:o + SC],
                        ag_out[2 * j + i, :, :, :].rearrange("t p s -> p t s"))

            # ---------------- weight / table loads (overlap the AllGather) ----------
            wq_sb = pp.tile([128, DT, LHEADS * HD], BF, name="wq_sb")
            nc.gpsimd.dma_start(wq_sb[:], wqp[:].rearrange("(t p) m -> p t m", p=128))
            wkv_sb = pp.tile([128, DT, 2 * HD], BF, name="wkv_sb")
            nc.gpsimd.dma_start(wkv_sb[:], wkv[:].rearrange("(t p) m -> p t m", p=128))

            # cos4: cosT replicated on every 32-partition block
            # sin4s: signed sin table: rows 32b..32b+31 = (-1)^(b+1) * sinT
            cos4 = pp.tile([128, SEQ], BF, name="cos4")
            sin4s = pp.tile([128, SEQ], BF, name="sin4s")
            for b in range(4):
                nc.gpsimd.dma_start(cos4[32 * b:32 * b + 32, :], cosT[:])
                nc.gpsimd.dma_start(sin4s[32 * b:32 * b + 32, :], sinT[:])
            for b in (0, 2):   # negate blocks 0 and 2 (the "real out" rows)
                nc.vector.tensor_scalar_mul(sin4s[32 * b:32 * b + 32, :],
                                            sin4s[32 * b:32 * b + 32, :], -1.0)

            # first 8 k-tiles of wo: dedicated slot
            woA = pp.tile([128, DT // 2, DIM], BF, name="woA")
            for g in range(2):
                nc.gpsimd.dma_start(
                    woA[:, 4 * g:4 * g + 4, :],
                    wo[512 * g:512 * g + 512, :].rearrange("(t p) n -> p t n",
                                                           p=128))

            # 0/1 causal patterns for diagonal-band tiles, paired for the
            # [128, 1024] two-head exp tiles: [pat_t | pat_t]
            patp = []
            for t in range(4):
                pat = pp.tile([128, 2, CH], BF, name=f"pat{t}")
                nc.gpsimd.memset(pat[:], 1.0)
                for half in range(2):
                    nc.gpsimd.affine_select(
                        out=pat[:, half, :], in_=pat[:, half, :],
                        compare_op=mybir.AluOpType.is_ge, fill=0.0,
                        base=-128 * t, channel_multiplier=-1, pattern=[[1, CH]],
                    )
                patp.append(pat)

            # ---------------- phase 2: projections + RoPE ----------------
            qT = pp.tile([128, 2, SEQ], BF, name="qT")   # [pair]; even head 0-63, odd 64-127
            kT = pp.tile([128, SEQ], BF, name="kT")      # rows 0-63 kT; 64-127 duplicate
            vT = pp.tile([64, SEQ], BF, name="vT")
            v_sb = pp.tile([128, KT, 2 * HD], BF, name="v_sb")
            nc.gpsimd.memset(v_sb[:, :, HD:2 * HD], 1.0)

            def rope(ps, nrows, j, dst):
                # dst rows [32b, 32b+32) for even b are "real" outputs, odd b "imag";
                # ps rows: same blocks hold q_r (even) / q_i (odd) pre-rotation.
                # out = ps*cos4 + cross, cross[b] = ps[b^1 block] * sin4s[b]
                sl = slice(CH * j, CH * j + CH)
                ta = wp.tile([128, CH], F32, tag="ropeA", bufs=2, name="ta")
                cr = wp.tile([128, CH], F32, tag="ropeB", bufs=2, name="cr")
                nc.vector.tensor_tensor(ta[0:nrows, :], ps[0:nrows, :],
                                        cos4[0:nrows, sl], MUL)
                for b in range(0, nrows, 32):
                    o = 32 if b % 64 == 0 else -32   # partner block offset
                    nc.vector.tensor_tensor(cr[b:b + 32, :],
                                            ps[b + o:b + o + 32, :],
                                            sin4s[b:b + 32, sl], MUL)
                nc.vector.tensor_tensor(dst, ta[0:nrows, :], cr[0:nrows, :], ADD)

            def qproj(pair, j):
                psq = psp.tile([128, CH], F32, tag="ps", bufs=4, name="psq")
                for dt in range(DT):
                    nc.tensor.matmul(
                        psq[:], wq_sb[:, dt, 128 * pair:128 * pair + 128],
                        xt[:, dt, CH * j:CH * j + CH],
                        start=(dt == 0), stop=(dt == DT - 1))
                rope(psq, 128, j, qT[0:128, pair, CH * j:CH * j + CH])

            for j in range(NCH):
                pskv = psp.tile([128, CH], F32, tag="ps", bufs=4, name="pskv")
                for dt in range(DT):
                    nc.tensor.matmul(pskv[:], wkv_sb[:, dt, :],
                                     xt[:, dt, CH * j:CH * j + CH],
                                     start=(dt == 0), stop=(dt == DT - 1))
                rope(pskv, 64, j, kT[0:64, CH * j:CH * j + CH])
                nc.vector.tensor_copy(vT[:, CH * j:CH * j + CH], pskv[64:128, :])

            # duplicate kT rows 0-63 -> 64-127 (for row-packed dual matmuls)
            nc.vector.tensor_copy(kT[64:128, :], kT[0:64, :])
            # v natural layout via PE transposes of vT
            for kt in range(KT):
                vtp = psp.tile([128, 64], BF, tag="ps", bufs=4, name="vtp")
                nc.tensor.transpose(vtp[:], vT[:, 128 * kt:128 * kt + 128],
                                    ident[0:64, 0:64])
                nc.vector.tensor_copy(v_sb[:, kt, 0:HD], vtp[:])

            for j in range(NCH):
                qproj(0, j)

            # ---------------- phase 3: attention ----------------
            attnT = pp.tile([128, 2, SEQ], BF, name="attnT")

            def attention(pair, j):
                nkt = 4 * j + 4
                pso0 = psp.tile([2 * HD, CH], F32, tag="ps", bufs=4, name="pso0")
                pso1 = psp.tile([2 * HD, CH], F32, tag="ps", bufs=4, name="pso1")
                qsl = slice(CH * j, CH * j + CH)
                for kt in range(nkt):
                    ks = slice(128 * kt, 128 * kt + 128)
                    sp = psp.tile([128, 2 * CH], F32, tag="spair", bufs=2, name="sp")
                    nc.tensor.matmul(sp[:, 0:CH], kT[0:64, ks], qT[0:64, pair, qsl],
                                     start=True, stop=True)
                    nc.tensor.matmul(sp[:, CH:2 * CH], kT[64:128, ks],
                                     qT[64:128, pair, qsl], start=True, stop=True)
                    ep = wp.tile([128, 2 * CH], BF, tag="exps", bufs=4, name="ep")
                    nc.scalar.activation(ep[:], sp[:],
                                         mybir.ActivationFunctionType.Exp,
                                         scale=0.125)
                    if kt >= 4 * j:
                        pat = patp[kt - 4 * j]
                        nc.vector.tensor_tensor(ep[:], ep[:],
                                                pat[:].rearrange("p a c -> p (a c)"),
                                                MUL)
                    nc.tensor.matmul(pso0[:], v_sb[:, kt, :], ep[:, 0:CH],
                                     start=(kt == 0), stop=(kt == nkt - 1))
                    nc.tensor.matmul(pso1[:], v_sb[:, kt, :], ep[:, CH:2 * CH],
                                     start=(kt == 0), stop=(kt == nkt - 1))
                for h, pso in ((0, pso0), (1, pso1)):
                    bc = wp.tile([64, CH], F32, tag="bcast", bufs=2, name="bc")
                    nc.vector.tensor_copy(bc[:], pso[HD:2 * HD, :])
                    rc = wp.tile([64, CH], F32, tag="rcp", bufs=2, name="rc")
                    nc.vector.reciprocal_approx_fast(out=rc[:], in_=bc[:])
                    nc.vector.tensor_tensor(
                        attnT[64 * h:64 * h + 64, pair, qsl],
                        pso[0:HD, :], rc[:], MUL)

            # interleave: attention(pair 0) with q projections of pair 1
            for j in range(NCH):
                attention(0, j)
                qproj(1, j)
            # pair-0 AllToAll overlaps pair-1 attention
            for dst in range(NC_CORES):
                nc.sync.dma_start(a2a_in0[dst, :, :],
                                  attnT[:, 0, SC * dst:SC * dst + SC])
            nc.gpsimd.collective_compute(
                "AllToAll", mybir.AluOpType.bypass,
                replica_groups=groups, ins=[a2a_in0.opt()], outs=[a2a_out0.opt()],
            )
            # second 8 k-tiles of wo: reuses xt's slot (dead after last qproj)
            woB = bigp.tile([128, DT // 2, DIM], BF, tag="bigslot", name="woB")
            for g in range(2):
                nc.gpsimd.dma_start(
                    woB[:, 4 * g:4 * g + 4, :],
                    wo[1024 + 512 * g:1024 + 512 * g + 512, :]
                    .rearrange("(t p) n -> p t n", p=128))
            for j in range(NCH):
                attention(1, j)

            # ---------------- phase 4: AllToAll + output projection ----------------
            for dst in range(NC_CORES):
                nc.sync.dma_start(a2a_in1[dst, :, :],
                                  attnT[:, 1, SC * dst:SC * dst + SC])
            nc.gpsimd.collective_compute(
                "AllToAll", mybir.AluOpType.bypass,
                replica_groups=groups, ins=[a2a_in1.opt()], outs=[a2a_out1.opt()],
            )
            a2a_sb = pp.tile([128, 2 * NC_CORES, SC], BF, name="a2a_sb")
            for src in range(NC_CORES):
                for pair, a2a_out_p in ((0, a2a_out0), (1, a2a_out1)):
                    nc.sync.dma_start(
                        a2a_sb[:, 2 * src + pair, :],
                        a2a_out_p[src, :, :])

            for qt in range(2):
                for nch in range(NCH):
                    psf = psp.tile([128, CH], F32, tag="ps", bufs=4, name="psf")
                    nsl = slice(CH * nch, CH * nch + CH)
                    for g in range(2 * NC_CORES):
                        w_ap = (woA[:, g, nsl] if g < DT // 2
                                else woB[:, g - DT // 2, nsl])
                        nc.tensor.matmul(psf[:], a2a_sb[:, g, 128 * qt:128 * qt + 128],
                                         w_ap,
                                         start=(g == 0), stop=(g == 2 * NC_CORES - 1))
                    osb = wp.tile([128, CH], F32, tag="osb", bufs=2, name="osb")
                    nc.vector.tensor_copy(osb[:], psf[:])
                    nc.sync.dma_start(out[128 * qt:128 * qt + 128, nsl], osb[:])

    nc.finalize()
    return nc


def _get_nc():
    if "nc" not in _CACHE:
        _CACHE["nc"] = _build_nc()
    return _CACHE["nc"]


_PERM = np.concatenate([np.arange(0, HD, 2), np.arange(1, HD, 2)])  # de-interleave


def _shard(inputs):
    x = np.ascontiguousarray(inputs["x"][0].astype(np.float32))          # [S, D]
    wq, wk, wv = inputs["wq"], inputs["wk"], inputs["wv"]
    wo = np.ascontiguousarray(inputs["wo"].astype(np.float32))
    cosT = np.ascontiguousarray(inputs["freqs_cos"].T.astype(np.float32))
    sinT = np.ascontiguousarray(inputs["freqs_sin"].T.astype(np.float32))
    in_maps = []
    for c in range(NC_CORES):
        wq_c = wq[:, LHEADS * HD * c:LHEADS * HD * (c + 1)].reshape(DIM, LHEADS, HD)
        wqp = np.ascontiguousarray(wq_c[:, :, _PERM].reshape(DIM, LHEADS * HD)
                                   .astype(np.float32))
        wk_c = wk[:, HD * c:HD * (c + 1)][:, _PERM]
        wv_c = wv[:, HD * c:HD * (c + 1)]
        wkv = np.ascontiguousarray(
            np.concatenate([wk_c, wv_c], axis=1).astype(np.float32))
        in_maps.append({
            "x_sl": np.ascontiguousarray(x[SC * c:SC * (c + 1), :]),
            "wqp": wqp,
            "wkv": wkv,
            "wo": wo,
            "cosT": cosT,
            "sinT": sinT,
        })
    return in_maps


def kernel(**inputs):
    from concourse.bass_utils import run_bass_kernel_spmd

    nc = _get_nc()
    in_maps = _shard(inputs)
    res = run_bass_kernel_spmd(nc, in_maps, core_ids=list(range(NC_CORES)))
    out = np.concatenate([res.results[c]["out"] for c in range(NC_CORES)], axis=0)
    return out[None].astype(np.float32)


# revision 12
# speedup vs baseline: 1.1067x; 1.0083x over previous
"""Tensor-parallel GQA attention forward for one TRN2 chip (8 NeuronCores).

Strategy (8-way tensor parallel over heads):
  - each core owns 4 q-heads + 1 kv-head (wq/wk/wv column-sharded, host side)
  - x is transposed on-device: each core PE-transposes its 256-row slice of x
    (cast to bf16) and an AllGather assembles the full xT on every core
  - projections produce qT/kT (head_dim on partitions) and v (natural layout)
    directly in the layouts the attention matmuls want; RoPE is applied in a
    de-interleaved head-dim ordering (dot products are permutation invariant)
  - scores are computed transposed (S^T[k, q]) so exp runs straight out of
    PSUM; softmax denominators come for free as a 65th column of ones in the
    PV matmul; causal masking = skipping k-tiles above the diagonal plus a
    0/1 pattern multiply on the 4 diagonal-band tiles per chunk
  - an AllToAll flips head-sharded attnT to sequence-sharded, each core then
    computes its 256-row slice of the output projection against full wo
  - compute dtype bf16 (fp32 PSUM accumulation), output fp32
"""

import numpy as np

NC_CORES = 8
SEQ = 2048
DIM = 2048
HD = 64            # head dim
LHEADS = 4         # q heads per core
SC = SEQ // NC_CORES   # 256: sequence rows per core (transpose shard / output shard)
CH = 512           # q-chunk width for attention
NCH = SEQ // CH    # 4
KT = SEQ // 128    # 16 k-tiles
DT = DIM // 128    # 16 d-tiles

_CACHE = {}


def _build_nc():
    import concourse.bass as bass
    import concourse.mybir as mybir
    import concourse.tile as tile
    from concourse import bacc
    from concourse.masks import make_identity

    BF = mybir.dt.bfloat16
    F32 = mybir.dt.float32
    MUL = mybir.AluOpType.mult
    ADD = mybir.AluOpType.add

    nc = bacc.Bacc("TRN2", target_bir_lowering=False, debug=False,
                   num_devices=NC_CORES)

    # ---- external I/O (per-core shards) ----
    x_sl = nc.dram_tensor("x_sl", [SC, DIM], F32, kind="ExternalInput")
    wqp = nc.dram_tensor("wqp", [DIM, LHEADS * HD], F32, kind="ExternalInput")
    wkv = nc.dram_tensor("wkv", [DIM, 2 * HD], F32, kind="ExternalInput")
    wo = nc.dram_tensor("wo", [DIM, DIM], F32, kind="ExternalInput")
    cosT = nc.dram_tensor("cosT", [32, SEQ], F32, kind="ExternalInput")
    sinT = nc.dram_tensor("sinT", [32, SEQ], F32, kind="ExternalInput")
    out = nc.dram_tensor("out", [SC, DIM], F32, kind="ExternalOutput")

    groups = [list(range(NC_CORES))]

    with tile.TileContext(nc) as tc:
        # DRAM bounce buffers for collectives
        ag_in, _ = tc.tile([DT, 128, SC], BF, space=bass.MemorySpace.DRAM,
                           name="ag_in")
        ag_out, _ = tc.tile([NC_CORES, DT, 128, SC], BF,
                            space=bass.MemorySpace.DRAM,
                            addr_space="Shared", name="ag_out")
        a2a_in0, _ = tc.tile([NC_CORES, 128, SC], BF,
                             space=bass.MemorySpace.DRAM, name="a2a_in0")
        a2a_out0, _ = tc.tile([NC_CORES, 128, SC], BF,
                              space=bass.MemorySpace.DRAM,
                              addr_space="Shared", name="a2a_out0")
        a2a_in1, _ = tc.tile([NC_CORES, 128, SC], BF,
                             space=bass.MemorySpace.DRAM, name="a2a_in1")
        a2a_out1, _ = tc.tile([NC_CORES, 128, SC], BF,
                              space=bass.MemorySpace.DRAM,
                              addr_space="Shared", name="a2a_out1")

        with tc.tile_pool(name="big", bufs=1) as bigp, \
             tc.tile_pool(name="persist", bufs=1) as pp, \
             tc.tile_pool(name="work", bufs=2) as wp, \
             tc.tile_pool(name="psum", bufs=2, space="PSUM") as psp:

            # ---------------- phase 0+1: x transpose + AllGather first ----------------
            # (critical path: everything waits on the gathered xT, so the x
            # slice load, PE transposes, bounce store and collective go ahead
            # of every weight load in the DMA queues)
            ident = pp.tile([128, 128], BF, name="ident")
            make_identity(nc, ident[:])

            xsl_bf = pp.tile([128, 2, DIM], BF, name="xsl_bf")
            nc.gpsimd.dma_start(xsl_bf[:], x_sl[:].rearrange("(t p) d -> p t d", p=128))

            xTc = pp.tile([128, DT, SC], BF, name="xTc")
            for pt in range(2):
                for j in range(DT):
                    trp = psp.tile([128, 128], BF, tag="ps", bufs=4, name="trp")
                    nc.tensor.transpose(trp[:], xsl_bf[:, pt, 128 * j:128 * j + 128],
                                        ident[:])
                    nc.vector.tensor_copy(xTc[:, j, 128 * pt:128 * pt + 128], trp[:])
            nc.gpsimd.dma_start(ag_in[:].rearrange("t p s -> p t s"), xTc[:])
            nc.gpsimd.collective_compute(
                "AllGather", mybir.AluOpType.bypass,
                replica_groups=groups, ins=[ag_in.opt()], outs=[ag_out.opt()],
            )
            xt = bigp.tile([128, DT, SEQ], BF, tag="bigslot", name="xt")
            for j in range(NCH):   # per-chunk loads so projections start early
                for i in range(2):
                    o = CH * j + SC * i
                    nc.sync.dma_start(
                        xt[:, :, o:o + SC],
                        ag_out[2 * j + i, :, :, :].rearrange("t p s -> p t s"))

            # ---------------- weight / table loads (overlap the AllGather) ----------
            wq_sb = pp.tile([128, DT, LHEADS * HD], BF, name="wq_sb")
            nc.gpsimd.dma_start(wq_sb[:], wqp[:].rearrange("(t p) m -> p t m", p=128))
            wkv_sb = pp.tile([128, DT, 2 * HD], BF, name="wkv_sb")
            nc.gpsimd.dma_start(wkv_sb[:], wkv[:].rearrange("(t p) m -> p t m", p=128))

            # cos4: cosT replicated on every 32-partition block
            # sin4s: signed sin table: rows 32b..32b+31 = (-1)^(b+1) * sinT
            cos4 = pp.tile([128, SEQ], BF, name="cos4")
            sin4s = pp.tile([128, SEQ], BF, name="sin4s")
            for b in range(4):
                nc.gpsimd.dma_start(cos4[32 * b:32 * b + 32, :], cosT[:])
                nc.gpsimd.dma_start(sin4s[32 * b:32 * b + 32, :], sinT[:])
            for b in (0, 2):   # negate blocks 0 and 2 (the "real out" rows)
                nc.vector.tensor_scalar_mul(sin4s[32 * b:32 * b + 32, :],
                                            sin4s[32 * b:32 * b + 32, :], -1.0)

            # first 8 k-tiles of wo: dedicated slot
            woA = pp.tile([128, DT // 2, DIM], BF, name="woA")
            for g in range(2):
                nc.gpsimd.dma_start(
                    woA[:, 4 * g:4 * g + 4, :],
                    wo[512 * g:512 * g + 512, :].rearrange("(t p) n -> p t n",
                                                           p=128))

            # 0/1 causal patterns for diagonal-band tiles, paired for the
            # [128, 1024] two-head exp tiles: [pat_t | pat_t]
            patp = []
            for t in range(4):
                pat = pp.tile([128, 2, CH], BF, name=f"pat{t}")
                nc.gpsimd.memset(pat[:], 1.0)
                for half in range(2):
                    nc.gpsimd.affine_select(
                        out=pat[:, half, :], in_=pat[:, half, :],
                        compare_op=mybir.AluOpType.is_ge, fill=0.0,
                        base=-128 * t, channel_multiplier=-1, pattern=[[1, CH]],
                    )
                patp.append(pat)

            # ---------------- phase 2: projections + RoPE ----------------
            qT = pp.tile([128, 2, SEQ], BF, name="qT")   # [pair]; even head 0-63, odd 64-127
            kT = pp.tile([128, SEQ], BF, name="kT")      # rows 0-63 kT; 64-127 duplicate
            vT = pp.tile([64, SEQ], BF, name="vT")
            v_sb = pp.tile([128, KT, 2 * HD], BF, name="v_sb")
            nc.gpsimd.memset(v_sb[:, :, HD:2 * HD], 1.0)

            def rope(ps, nrows, j, dst):
                # dst rows [32b, 32b+32) for even b are "real" outputs, odd b "imag";
                # ps rows: same blocks hold q_r (even) / q_i (odd) pre-rotation.
                # out = ps*cos4 + cross, cross[b] = ps[b^1 block] * sin4s[b]
                sl = slice(CH * j, CH * j + CH)
                ta = wp.tile([128, CH], F32, tag="ropeA", bufs=2, name="ta")
                cr = wp.tile([128, CH], F32, tag="ropeB", bufs=2, name="cr")
                nc.vector.tensor_tensor(ta[0:nrows, :], ps[0:nrows, :],
                                        cos4[0:nrows, sl], MUL)
                for b in range(0, nrows, 32):
                    o = 32 if b % 64 == 0 else -32   # partner block offset
                    nc.vector.tensor_tensor(cr[b:b + 32, :],
                                            ps[b + o:b + o + 32, :],
                                            sin4s[b:b + 32, sl], MUL)
                nc.vector.tensor_tensor(dst, ta[0:nrows, :], cr[0:nrows, :], ADD)

            def qproj(pair, j):
                psq = psp.tile([128, CH], F32, tag="ps", bufs=4, name="psq")
                for dt in range(DT):
                    nc.tensor.matmul(
                        psq[:], wq_sb[:, dt, 128 * pair:128 * pair + 128],
                        xt[:, dt, CH * j:CH * j + CH],
                        start=(dt == 0), stop=(dt == DT - 1))
                rope(psq, 128, j, qT[0:128, pair, CH * j:CH * j + CH])

            for j in range(NCH):
                pskv = psp.tile([128, CH], F32, tag="ps", bufs=4, name="pskv")
                for dt in range(DT):
                    nc.tensor.matmul(pskv[:], wkv_sb[:, dt, :],
                                     xt[:, dt, CH * j:CH * j + CH],
                                     start=(dt == 0), stop=(dt == DT - 1))
                rope(pskv, 64, j, kT[0:64, CH * j:CH * j + CH])
                nc.vector.tensor_copy(vT[:, CH * j:CH * j + CH], pskv[64:128, :])

            # duplicate kT rows 0-63 -> 64-127 (for row-packed dual matmuls)
            nc.vector.tensor_copy(kT[64:128, :], kT[0:64, :])
            # v natural layout via PE transposes of vT
            for kt in range(KT):
                vtp = psp.tile([128, 64], BF, tag="ps", bufs=4, name="vtp")
                nc.tensor.transpose(vtp[:], vT[:, 128 * kt:128 * kt + 128],
                                    ident[0:64, 0:64])
                nc.vector.tensor_copy(v_sb[:, kt, 0:HD], vtp[:])

            for j in range(NCH):
                qproj(0, j)

            # ---------------- phase 3: attention ----------------
            attnT = pp.tile([128, 2, SEQ], BF, name="attnT")

            def attention(pair, j):
                nkt = 4 * j + 4
                pso0 = psp.tile([2 * HD, CH], F32, tag="ps", bufs=4, name="pso0")
                pso1 = psp.tile([2 * HD, CH], F32, tag="ps", bufs=4, name="pso1")
                qsl = slice(CH * j, CH * j + CH)
                for kt in range(nkt):
                    ks = slice(128 * kt, 128 * kt + 128)
                    sp = psp.tile([128, 2 * CH], F32, tag="spair", bufs=2, name="sp")
                    nc.tensor.matmul(sp[:, 0:CH], kT[0:64, ks], qT[0:64, pair, qsl],
                                     start=True, stop=True)
                    nc.tensor.matmul(sp[:, CH:2 * CH], kT[64:128, ks],
                                     qT[64:128, pair, qsl], start=True, stop=True)
                    ep = wp.tile([128, 2 * CH], BF, tag="exps", bufs=4, name="ep")
                    nc.scalar.activation(ep[:], sp[:],
                                         mybir.ActivationFunctionType.Exp,
                                         scale=0.125)
                    if kt >= 4 * j:
                        pat = patp[kt - 4 * j]
                        nc.vector.tensor_tensor(ep[:], ep[:],
                                                pat[:].rearrange("p a c -> p (a c)"),
                                                MUL)
                    nc.tensor.matmul(pso0[:], v_sb[:, kt, :], ep[:, 0:CH],
                                     start=(kt == 0), stop=(kt == nkt - 1))
                    nc.tensor.matmul(pso1[:], v_sb[:, kt, :], ep[:, CH:2 * CH],
                                     start=(kt == 0), stop=(kt == nkt - 1))
                for h, pso in ((0, pso0), (1, pso1)):
                    bc = wp.tile([64, CH], F32, tag="bcast", bufs=2, name="bc")
                    nc.vector.tensor_copy(bc[:], pso[HD:2 * HD, :])
                    rc = wp.tile([64, CH], F32, tag="rcp", bufs=2, name="rc")
                    nc.vector.reciprocal_approx_fast(out=rc[:], in_=bc[:])
                    nc.vector.tensor_tensor(
                        attnT[64 * h:64 * h + 64, pair, qsl],
                        pso[0:HD, :], rc[:], MUL)

            # interleave: attention(pair 0) with q projections of pair 1
            for j in range(NCH):
                attention(0, j)
                qproj(1, j)
            # pair-0 AllToAll overlaps pair-1 attention
            for dst in range(NC_CORES):
                nc.gpsimd.dma_start(a2a_in0[dst, :, :],
                                    attnT[:, 0, SC * dst:SC * dst + SC])
            nc.gpsimd.collective_compute(
                "AllToAll", mybir.AluOpType.bypass,
                replica_groups=groups, ins=[a2a_in0.opt()], outs=[a2a_out0.opt()],
            )
            a2a_sb = pp.tile([128, 2 * NC_CORES, SC], BF, name="a2a_sb")
            for src in range(NC_CORES):
                nc.sync.dma_start(a2a_sb[:, 2 * src, :], a2a_out0[src, :, :])
            # second 8 k-tiles of wo: reuses xt's slot (dead after last qproj)
            woB = bigp.tile([128, DT // 2, DIM], BF, tag="bigslot", name="woB")
            for g in range(2):
                nc.gpsimd.dma_start(
                    woB[:, 4 * g:4 * g + 4, :],
                    wo[1024 + 512 * g:1024 + 512 * g + 512, :]
                    .rearrange("(t p) n -> p t n", p=128))
            for j in range(NCH):
                attention(1, j)

            # ---------------- phase 4: AllToAll + output projection ----------------
            for dst in range(NC_CORES):
                nc.gpsimd.dma_start(a2a_in1[dst, :, :],
                                    attnT[:, 1, SC * dst:SC * dst + SC])
            nc.gpsimd.collective_compute(
                "AllToAll", mybir.AluOpType.bypass,
                replica_groups=groups, ins=[a2a_in1.opt()], outs=[a2a_out1.opt()],
            )
            for src in range(NC_CORES):
                nc.sync.dma_start(a2a_sb[:, 2 * src + 1, :], a2a_out1[src, :, :])

            # accumulate pair-0 contributions first: those only need A2A #1,
            # so the first half of each psf overlaps the second AllToAll
            gseq = [2 * src for src in range(NC_CORES)] + \
                   [2 * src + 1 for src in range(NC_CORES)]
            for qt in range(2):
                for nch in range(NCH):
                    psf = psp.tile([128, CH], F32, tag="ps", bufs=4, name="psf")
                    nsl = slice(CH * nch, CH * nch + CH)
                    for i, g in enumerate(gseq):
                        w_ap = (woA[:, g, nsl] if g < DT // 2
                                else woB[:, g - DT // 2, nsl])
                        nc.tensor.matmul(psf[:], a2a_sb[:, g, 128 * qt:128 * qt + 128],
                                         w_ap,
                                         start=(i == 0), stop=(i == 2 * NC_CORES - 1))
                    osb = wp.tile([128, CH], F32, tag="osb", bufs=2, name="osb")
                    nc.vector.tensor_copy(osb[:], psf[:])
                    nc.sync.dma_start(out[128 * qt:128 * qt + 128, nsl], osb[:])

    nc.finalize()
    return nc


def _get_nc():
    if "nc" not in _CACHE:
        _CACHE["nc"] = _build_nc()
    return _CACHE["nc"]


_PERM = np.concatenate([np.arange(0, HD, 2), np.arange(1, HD, 2)])  # de-interleave


def _shard(inputs):
    x = np.ascontiguousarray(inputs["x"][0].astype(np.float32))          # [S, D]
    wq, wk, wv = inputs["wq"], inputs["wk"], inputs["wv"]
    wo = np.ascontiguousarray(inputs["wo"].astype(np.float32))
    cosT = np.ascontiguousarray(inputs["freqs_cos"].T.astype(np.float32))
    sinT = np.ascontiguousarray(inputs["freqs_sin"].T.astype(np.float32))
    in_maps = []
    for c in range(NC_CORES):
        wq_c = wq[:, LHEADS * HD * c:LHEADS * HD * (c + 1)].reshape(DIM, LHEADS, HD)
        wqp = np.ascontiguousarray(wq_c[:, :, _PERM].reshape(DIM, LHEADS * HD)
                                   .astype(np.float32))
        wk_c = wk[:, HD * c:HD * (c + 1)][:, _PERM]
        wv_c = wv[:, HD * c:HD * (c + 1)]
        wkv = np.ascontiguousarray(
            np.concatenate([wk_c, wv_c], axis=1).astype(np.float32))
        in_maps.append({
            "x_sl": np.ascontiguousarray(x[SC * c:SC * (c + 1), :]),
            "wqp": wqp,
            "wkv": wkv,
            "wo": wo,
            "cosT": cosT,
            "sinT": sinT,
        })
    return in_maps


def kernel(**inputs):
    from concourse.bass_utils import run_bass_kernel_spmd

    nc = _get_nc()
    in_maps = _shard(inputs)
    res = run_bass_kernel_spmd(nc, in_maps, core_ids=list(range(NC_CORES)))
    out = np.concatenate([res.results[c]["out"] for c in range(NC_CORES)], axis=0)
    return out[None].astype(np.float32)
